# revision 40
# baseline (speedup 1.0000x reference)
"""NonLocalAttention Trainium2 kernel.

Reference computation (N=2, C=64, CR=32, H=W=96, HW=9216):
    e1  = PReLU(w1 @ inputa + b1)   # [N,32,HW]   (queries)
    e2  = PReLU(w2 @ inputb + b2)   # [N,32,HW]   (keys)
    asm = PReLU(wa @ inputa + ba)   # [N,64,HW]   (values)
    out = softmax(e1^T e2, axis=keys) @ asm^T + inputa

Sharding: 8 cores = 2 batches x 4 query-chunks of 2304 rows. Each core gets
its batch's full inputa/inputb (for keys/values), and writes a disjoint
[65, 2304] slice of unnormalized output (64 channels + softmax denominator);
the final division and residual add happen on the host (cheap: 1.2M flops).
No collectives.

Per-core kernel (flash-style, never materializes [HW,HW]). The steady state
is ScalarE-bound (exp of 21.2M scores at 1 elem/cycle/lane @1.2GHz ~ 140us
floor), so the structure keeps the ACT engine saturated end-to-end:

  - conv biases are folded into the matmuls via a ones-row at row 64 of the
    activations.  The conv weights for e1/e2 are REPLICATED 4x along their
    output dim (w1 additionally scaled by 1/4, exact in bf16): the conv psum
    comes out as 4 vertical replicas of the e-channels for free, and QK
    contracts K=128 over the replicas, summing 4 * (e1/4 * e2) = the exact
    score.  No big zero-fill memsets for e1/e2, no score rescale.
  - matmuls whose moving operand spans <128 partitions stream at HALF rate
    on this silicon; the K=65 prologue matmuls accept that (the ~4us it
    costs is less than what zero-padding xb/xq to 128 rows costs in memset
    serialization at the head).
  - a short burst of dummy warm-up matmuls overlaps the input DMA so the PE
    HAM clock-gate reaches 2.4 GHz before the real prologue begins.
  - attention uses the S^T = e2^T e1 orientation: keys land on the PSUM
    partition dim, so the PV matmul needs no transposes at all, and an
    all-ones 65th column in the value tiles makes the PV matmul emit the
    softmax denominator as PSUM row 64 for free.
  - scores are bounded (|s| <= 32 * max|e1| * max|e2| << 88) so exp needs
    no max-subtraction.
  - everything on the PE is bf16; PSUM accumulation stays fp32; the raw
    [65, nq] accumulator is copied to SBUF and DMA'd out per q-block.
  - PSUM: 2x3-bank double-buffered score groups + 2x1-bank po accumulators
    = 8 banks exactly.
  - input DMA is split across two queues (sync: xb/xq, gpsimd: xa) so the
    loads stream in parallel.
"""

import numpy as np

C = 64
CR = 32
K65 = 65  # contraction rows: 64 channels + ones row (bias)
HW = 9216
QCH = 2304  # query rows per core
NKT = HW // 128  # 72 key tiles
NCORES = 8
NWARM = 24  # dummy matmuls to lift the HAM clock gate during input DMA
QBLOCKS = [(0, 512), (512, 512), (1024, 512), (1536, 512), (2048, 256)]


def _ensure_ntff_hook():
    """Best-effort registration of the axon NTFF profile hook; the agent
    image's antenv package lacks axon_hooks, which would make any traced
    run crash on import instead of degrading."""
    import sys
    import types

    try:
        import antenv.axon_hooks  # noqa: F401

        return
    except ImportError:
        pass
    try:
        import antenv
        from trn_agent_boot.trn_boot import _ntff_profile_via_ctypes

        hook = _ntff_profile_via_ctypes("/opt/axon/libaxon_pjrt.so")
        mod = types.ModuleType("antenv.axon_hooks")
        _h = [hook]
        mod.get_axon_ntff_profile_hook = lambda: _h[0]
        mod.set_axon_ntff_profile_hook = lambda h: _h.__setitem__(0, h)
        sys.modules["antenv.axon_hooks"] = mod
        antenv.axon_hooks = mod
    except Exception:
        pass


def build_program(a1: float, a2: float, aa: float):
    import concourse.bacc as bacc
    import concourse.tile as tile
    from concourse import mybir

    f32 = mybir.dt.float32
    bf16 = mybir.dt.bfloat16
    AF = mybir.ActivationFunctionType

    nc = bacc.Bacc()
    # xa/xb arrive per-core ROTATED so this core's query chunk is cols
    # [0, QCH) -- softmax sums are key-order invariant, so rotating keys
    # is pure host-side relabeling and removes a redundant xq load.
    xa = nc.dram_tensor("xa", [K65, HW], bf16, kind="ExternalInput")
    xb = nc.dram_tensor("xb", [K65, HW], bf16, kind="ExternalInput")
    w1q = nc.dram_tensor("w1q", [K65, 128], bf16, kind="ExternalInput")
    w2q = nc.dram_tensor("w2q", [K65, 128], bf16, kind="ExternalInput")
    waq = nc.dram_tensor("waq", [K65, C], bf16, kind="ExternalInput")
    out = nc.dram_tensor("out", [C + 1, QCH], f32, kind="ExternalOutput")

    with tile.TileContext(nc) as tc:
        with (
            tc.tile_pool(name="consts", bufs=1) as consts,
            tc.tile_pool(name="big", bufs=1) as big,
            tc.tile_pool(name="ps", bufs=2, space="PSUM") as ps,
            tc.tile_pool(name="po", bufs=1, space="PSUM") as ps_o,
            tc.tile_pool(name="pv", bufs=1, space="PSUM") as ps_v,
            tc.tile_pool(name="pt", bufs=3) as ptile,
            tc.tile_pool(name="work", bufs=2) as work,
        ):
            # --- PE warm-up: junk matmuls with no DMA dependency ---------
            wdum = consts.tile([128, 512], bf16, tag="wdum")
            nc.vector.memset(wdum[:], 0.25)
            for w in range(NWARM):
                psw = ps.tile([128, 512], f32, tag="pss")
                nc.tensor.matmul(
                    psw[:], wdum[:, 0:128], wdum[:], start=True, stop=True
                )

            # --- weights ------------------------------------------------
            w1_sb = consts.tile([K65, 128], bf16, tag="w1")
            nc.sync.dma_start(w1_sb[:], w1q[:])
            w2_sb = consts.tile([K65, 128], bf16, tag="w2")
            nc.sync.dma_start(w2_sb[:], w2q[:])
            wa_sb = consts.tile([K65, C], bf16, tag="wa")
            nc.sync.dma_start(wa_sb[:], waq[:])

            # --- activations in, all on the gpsimd ring (measured much
            # faster than the sync ring); xa chunk 0 first (it is also the
            # query chunk), then xb (gates attention), then the rest of xa.
            xa_sb = big.tile([K65, HW], bf16, tag="xa")
            xb_sb = big.tile([K65, HW], bf16, tag="xb")
            chunks = [("a", 0), ("b", 0), ("b", 1), ("b", 2), ("b", 3),
                      ("a", 1), ("a", 2), ("a", 3)]
            for which, k in chunks:
                off = k * QCH
                src, dst = (xb, xb_sb) if which == "b" else (xa, xa_sb)
                nc.gpsimd.dma_start(
                    dst[:, off : off + QCH], src[:, off : off + QCH]
                )

            # --- v_aug tiles: [128, 65] bf16 per key tile, col 64 = ones -
            # v batches 0-1 run inside the prologue (they only need xa and
            # fill the xb DMA-wait stalls); batches 2-8 interleave into
            # block 0 of the attention loop, prelu on the otherwise-idle
            # DVE, hiding in the exp-bound steady state.
            v_all = big.tile([128, NKT * 65], bf16, tag="vall")
            v3 = v_all[:].rearrange("p (t c) -> p t c", c=65)
            nc.vector.memset(v3[:, :, 64:65], 1.0)

            def emit_v_group(grp):
                psv = ps_v.tile([128, 512], f32, tag="psv", name=f"psv{grp}")
                for j in range(8):
                    i = grp * 8 + j
                    nc.tensor.matmul(
                        psv[:, j * 64 : (j + 1) * 64],
                        xa_sb[:, i * 128 : (i + 1) * 128],
                        wa_sb[:],
                        start=(j == 0), stop=(j == 7),
                    )
                psv3 = psv[:].rearrange("p (t c) -> p t c", c=64)
                ya = work.tile([128, 512], f32, tag="ya", name=f"ya{grp}")
                ya3 = ya[:].rearrange("p (t c) -> p t c", c=64)
                nc.vector.tensor_scalar_mul(ya[:], psv[:], aa)
                nc.vector.tensor_max(
                    v3[:, grp * 8 : (grp + 1) * 8, 0:64], ya3[:], psv3[:]
                )

            # --- e1 / e2 prologue, 512-col micro-chunks over 7 psum slots
            # Before the attention starts these PSUM banks are free: slice
            # the two 3-bank score tiles into 6 single-bank slots and borrow
            # the po bank for a 7th (pool rotation hands them back to the
            # attention loop automatically).  Matmuls stream back-to-back;
            # prelus chase on ScalarE (3/4) and DVE (1/4).
            e1_sb = big.tile([128, QCH], bf16, tag="e1")
            e2_sb = big.tile([128, HW], bf16, tag="e2")
            pA = ps.tile([128, 1536], f32, tag="pss")
            pB = ps.tile([128, 1536], f32, tag="pss")
            pC = ps_o.tile([128, 512], f32, tag="po")
            slots = [(pA, 0), (pB, 0), (pA, 512), (pB, 512),
                     (pA, 1024), (pB, 1024), (pC, 0)]
            jobs = [("1", off, min(512, QCH - off)) for off in range(0, QCH, 512)]
            jobs += [("2", off, 512) for off in range(0, HW, 512)]
            for k, (which, off, cw) in enumerate(jobs):
                buf, c0 = slots[k % 7]
                w_sb, dst, al = (
                    (w1_sb, e1_sb, a1) if which == "1"
                    else (w2_sb, e2_sb, a2)
                )
                x_sb = xa_sb if which == "1" else xb_sb
                nc.tensor.matmul(
                    buf[:, c0 : c0 + cw], w_sb[:], x_sb[:, off : off + cw],
                    start=True, stop=True,
                )
                if k % 4 == 3:
                    ya = work.tile([128, 512], f32, tag="ya2", name=f"ya2_{k}")
                    nc.vector.tensor_scalar_mul(
                        ya[:, 0:cw], buf[:, c0 : c0 + cw], al
                    )
                    nc.vector.tensor_max(
                        dst[:, off : off + cw], ya[:, 0:cw], buf[:, c0 : c0 + cw]
                    )
                else:
                    nc.scalar.activation(
                        dst[:, off : off + cw], buf[:, c0 : c0 + cw],
                        AF.Prelu, alpha=al,
                    )
                if k == 13:
                    emit_v_group(0)
                elif k == 18:
                    emit_v_group(1)

            # --- attention: per q-block, loop key tiles ------------------
            # S^T psum batches 3 key tiles (3 banks) per exp op.
            for off, nq in QBLOCKS:
                kt_per_ps = 1536 // nq  # 3 at nq=512, 6 at nq=256
                po = ps_o.tile([C + 1, nq], f32, tag="po")
                for g in range(NKT // kt_per_ps):
                    pss = ps.tile([128, 1536], f32, tag="pss")
                    for j in range(kt_per_ps):
                        i = g * kt_per_ps + j
                        colb = j * nq * 4  # byte offset of this matmul
                        # row-tiled QK: each matmul contracts ONE 32-row
                        # replica quadrant (K=32) on its own PE row-tile;
                        # quadrant = psum bank so concurrent tiles never
                        # share a bank.  Score comes out as s/4 (e1 is
                        # 1/4-scaled); the exp's scale=4 restores it.
                        q32 = ((colb // 2048) % 3) * 32
                        nc.tensor.matmul(
                            pss[:, j * nq : (j + 1) * nq],
                            e2_sb[q32 : q32 + 32, i * 128 : (i + 1) * 128],
                            e1_sb[q32 : q32 + 32, off : off + nq],
                            start=(colb % 2048 == 0),
                            stop=((colb + nq * 4) % 2048 == 0),
                        )
                    pt = ptile.tile([128, 1536], bf16, tag="pt")
                    nc.scalar.activation(pt[:], pss[:], AF.Exp, scale=4.0)
                    if off == 0 and g % 2 == 0 and 2 <= g // 2 + 1 < NKT // 8:
                        emit_v_group(g // 2 + 1)  # v batch hides under exp
                    for j in range(kt_per_ps):
                        i = g * kt_per_ps + j
                        nc.tensor.matmul(
                            po[:],
                            v_all[:, i * 65 : (i + 1) * 65],
                            pt[:, j * nq : (j + 1) * nq],
                            start=(i == 0), stop=(i == NKT - 1),
                        )
                # ship raw accumulator; host divides by row 64 + residual
                osb = work.tile([C + 1, nq], f32, tag="osb")
                nc.vector.tensor_copy(osb[:], po[:])
                nc.gpsimd.dma_start(out[:, off : off + nq], osb[:])
    nc.finalize()
    return nc


def run(inputs: dict, trace: bool = False, tmpdir: str | None = None):
    """Build, compile and run on 8 cores; returns (output, BassKernelResults)."""
    _ensure_ntff_hook()
    from concourse.bass_utils import run_bass_kernel_spmd

    inputa = np.asarray(inputs["inputa"], dtype=np.float32)
    inputb = np.asarray(inputs["inputb"], dtype=np.float32)
    w1 = np.asarray(inputs["w1"], dtype=np.float32)
    b1 = np.asarray(inputs["b1"], dtype=np.float32)
    w2 = np.asarray(inputs["w2"], dtype=np.float32)
    b2 = np.asarray(inputs["b2"], dtype=np.float32)
    wa = np.asarray(inputs["wa"], dtype=np.float32)
    ba = np.asarray(inputs["ba"], dtype=np.float32)
    a1 = float(np.asarray(inputs["a1"]).reshape(-1)[0])
    a2 = float(np.asarray(inputs["a2"]).reshape(-1)[0])
    aa = float(np.asarray(inputs["aa"]).reshape(-1)[0])

    N, Cc, H, W = inputa.shape
    assert (N, Cc, H * W) == (2, C, HW), inputa.shape
    chunks_per_batch = NCORES // N  # 4

    import ml_dtypes

    bf = ml_dtypes.bfloat16

    def aug65(x):
        """[64, n] -> [65, n] bf16 with a ones row at 64."""
        p = np.empty((K65, x.shape[1]), np.float32)
        p[:C] = x
        p[C] = 1.0
        return p.astype(bf)

    def wrep(wt, b, scale, reps, rows):
        """[64, m] weights + bias -> [rows, m*reps] bf16, zero rows 65.."""
        p = np.zeros((rows, wt.shape[1]), np.float32)
        p[:C] = wt
        p[C] = b
        p *= scale
        return np.tile(p, (1, reps)).astype(bf)

    w1q = wrep(w1.T, b1, 0.25, 4, K65)
    w2q = wrep(w2.T, b2, 1.0, 4, K65)
    waq = wrep(wa.T, ba, 1.0, 1, K65)

    xa_n = inputa.reshape(N, C, HW)
    xb_n = inputb.reshape(N, C, HW)
    xa_aug = [aug65(xa_n[b]) for b in range(N)]
    xb_aug = [aug65(xb_n[b]) for b in range(N)]

    in_maps = []
    for core in range(NCORES):
        b, chunk = divmod(core, chunks_per_batch)
        # rotate the token dim so this core's query chunk is cols [0, QCH);
        # key order is irrelevant to the softmax sums, so xb rotates the
        # same way and nothing else changes.
        in_maps.append(
            {
                "xa": np.roll(xa_aug[b], -chunk * QCH, axis=1),
                "xb": np.roll(xb_aug[b], -chunk * QCH, axis=1),
                "w1q": w1q,
                "w2q": w2q,
                "waq": waq,
            }
        )

    nc = build_program(a1, a2, aa)
    res = run_bass_kernel_spmd(
        nc, in_maps, list(range(NCORES)), trace=trace, tmpdir=tmpdir
    )

    out = np.empty((N, C, HW), np.float32)
    for core in range(NCORES):
        b, chunk = divmod(core, chunks_per_batch)
        raw = res.results[core]["out"]  # [65, QCH]: 64 ch + denominator
        sl = np.s_[:, chunk * QCH : (chunk + 1) * QCH]
        out[b][sl[0], sl[1]] = raw[:C] / raw[C] + xa_n[b][sl[0], sl[1]]
    return out.reshape(N, C, H, W), res


def kernel(**inputs) -> np.ndarray:
    out, _ = run(inputs, trace=False)
    return out
